# revision 12
# baseline (speedup 1.0000x reference)
"""Allegro GNN message-passing kernel for 8 Trainium2 NeuronCores.

Strategy (per spec sharding hint): edges sharded data-parallel across the 8
cores, sorted by sender and split at node boundaries so every node's outgoing
edges live on exactly one core; small weights replicated. Within a core,
node-runs are bin-packed into 512-edge tiles (<=63 real nodes per tile +
reserved dummy slot 63) so the sender segment-sum is tile-local: it becomes
PE-transposes + indicator matmuls through node space. The receiver
segment-sum is accumulated on device into a per-core [79,128] node grid
(onehot outer-product matmuls); the host sums the 8 partial grids (the
psum/unshard step) and adds per-species energies.

Per-edge scalar chains (distance, envelope, bessel, 1/d) are computed
column-packed [128, ncols] once per core - 128x fewer vector-engine columns
than row layout. All matmuls bf16 with fp32 PSUM accumulation.

kernel(**inputs) takes FULL (unsharded) numpy inputs, returns the FULL
(10000, 1) float32 output. Self-contained: shapes/constants hardcoded.
"""
import numpy as np

N_NODES = 10000
MUL = 32
N_RBF = 8
NCORES = 8
T = 512          # edges per tile
CH = 4           # chunks of 128 per tile
NN = 64          # node slots per tile (63 real + dummy slot 63)
NHI = 79         # ceil(10000/128) coarse receiver bins
DUMMY_RLO = 16.0  # dummy edges -> node 78*128+16 = 10000 >= N -> dropped
DUMMY_RHI = 78.0

SQ2 = float(np.sqrt(2.0))
SQ3 = float(np.sqrt(3.0))


def _bf16_dt():
    return np.float16


# ---------------------------------------------------------------------------
# Host-side sharding prep (index manipulation / packing only)
# ---------------------------------------------------------------------------
def _pack(vectors, senders, receivers, node_emb):
    bf = _bf16_dt()
    order = np.argsort(senders, kind='stable')
    s_sorted = senders[order]
    node_start = np.searchsorted(s_sorted, np.arange(N_NODES + 1))

    core_tiles = []
    for c in range(NCORES):
        nlo = (N_NODES * c) // NCORES
        nhi = (N_NODES * (c + 1)) // NCORES
        tiles = []
        cur_cnt, cur_slots, cur_e0 = 0, 0, int(node_start[nlo])
        for n in range(nlo, nhi):
            deg = int(node_start[n + 1] - node_start[n])
            if deg == 0:
                continue
            if cur_cnt + deg > T or cur_slots == NN - 1:
                tiles.append((cur_e0, cur_cnt))
                cur_e0 = int(node_start[n])
                cur_cnt, cur_slots = 0, 0
            cur_cnt += deg
            cur_slots += 1
        if cur_cnt:
            tiles.append((cur_e0, cur_cnt))
        core_tiles.append(tiles)

    TPC = max(len(t) for t in core_tiles)
    EPC = TPC * T
    Q = TPC * CH

    cores = []
    for c in range(NCORES):
        tiles = core_tiles[c]
        vec_pad = np.zeros((EPC, 3), np.float32)
        vec_pad[:, 0] = 1e-3
        slot_pad = np.full((EPC,), NN - 1, np.float32)
        rlo_pad = np.full((EPC,), DUMMY_RLO, np.float32)
        rhi_pad = np.full((EPC,), DUMMY_RHI, np.float32)
        zszr_pad = np.zeros((EPC, 64), np.float32)
        src_pad = np.full((EPC,), -1, np.int64)

        for t, (e0, cnt) in enumerate(tiles):
            idx = order[e0:e0 + cnt]
            base = t * T
            src_pad[base:base + cnt] = idx
            vec_pad[base:base + cnt] = vectors[idx]
            sl = s_sorted[e0:e0 + cnt]
            slot_pad[base:base + cnt] = np.searchsorted(
                np.unique(sl), sl).astype(np.float32)
            r = receivers[idx]
            rlo_pad[base:base + cnt] = (r % 128).astype(np.float32)
            rhi_pad[base:base + cnt] = (r // 128).astype(np.float32)
            zszr_pad[base:base + cnt, 0:32] = node_emb[senders[idx]]
            zszr_pad[base:base + cnt, 32:64] = node_emb[receivers[idx]]

        def colpack(flat2):   # [EPC, m] -> [128, m*Q], (j outer, q inner)
            m = flat2.shape[1]
            a = flat2.reshape(TPC, CH, 128, m)
            return a.transpose(2, 3, 0, 1).reshape(128, m * Q).copy()

        cores.append(dict(
            vcols=colpack(vec_pad).astype(np.float32),
            rlocols=colpack(rlo_pad[:, None]).astype(np.float32),
            rhicols=colpack(rhi_pad[:, None]).astype(np.float32),
            slotrow=slot_pad[None, :].astype(bf),
            zszr=np.ascontiguousarray(zszr_pad.T).astype(bf),
            _src=src_pad, _slot=slot_pad,
        ))
    return cores, TPC


def _prep_weights(inp):
    """Pre-scaled / fused weight tensors (bf16)."""
    bf = _bf16_dt()
    f = np.float32
    s = lambda W: (np.asarray(W, f) / np.sqrt(W.shape[0])).astype(f)
    n_irr = f(2 + 2 * 32)
    w = {}
    w['we0'] = s(inp['W_e0'])
    w['we1'] = s(inp['W_e1'])
    w['we2'] = s(inp['W_e2'])
    we3 = s(inp['W_e3'])
    w['we3a'], w['we3b'] = we3[0:128], we3[128:256]
    wv96 = np.zeros((64, 96), f)
    wvcol = np.asarray(inp['W_wvec'], f)[:, 0] / f(8.0)
    vinit = np.asarray(inp['W_vinit'], f) / n_irr
    for c in range(3):
        for m in range(MUL):
            wv96[:, c * 32 + m] = wvcol * vinit[m]
    w['wv96'] = wv96
    for l in range(2):
        w[f'ww{l}'] = (np.asarray(inp['W_w'][l], f) / f(8.0))
        wm0 = np.asarray(inp['W_m0'][l], f) / np.sqrt(f(96.0))
        w[f'wm0x{l}'] = wm0[0:64]
        wm0s = np.zeros((96, 64), f)
        for c in range(3):
            wm0s[c * 32:(c + 1) * 32] = wm0[64:96] / np.sqrt(f(3.0))
        w[f'wm0s{l}'] = wm0s
        w[f'wm1{l}'] = np.asarray(inp['W_m1'][l], f) / f(8.0)
    w['wm2'] = np.asarray(inp['W_m2'][0], f) / f(8.0)
    w['wm2ro'] = (np.asarray(inp['W_m2'][1], f) / f(8.0)
                  @ (np.asarray(inp['W_r0'], f) / f(8.0))
                  @ (np.asarray(inp['W_rout'], f) / f(8.0)))
    wv3 = np.zeros((96, 96), f)
    wvl = np.asarray(inp['W_V'][0], f) / np.sqrt(f(MUL))
    for c in range(3):
        wv3[c * 32:(c + 1) * 32, c * 32:(c + 1) * 32] = wvl
    w['wv3'] = wv3
    y1sel = np.zeros((3, 96), f)
    for c in range(3):
        y1sel[c, c * 32:(c + 1) * 32] = 1.0
    w['y1sel'] = y1sel
    w['ones64'] = np.ones((1, 64), f)
    w['ident'] = np.eye(128, dtype=f)
    consts = {
        'iota64': np.arange(64, dtype=f)[:, None].copy(),
        'c128': np.tile(np.arange(128, dtype=f)[None, :], (128, 1)),
        'c79': np.tile(np.arange(NHI, dtype=f)[None, :], (128, 1)),
        'id1f': np.ones((1, 1), f),
    }
    return {k: np.asarray(v, f).astype(bf) for k, v in w.items()}, consts


# ---------------------------------------------------------------------------
# Device program
# ---------------------------------------------------------------------------
def _build(TPC, eps, debug=False):
    import sys
    if '/opt/trn_rl_repo' not in sys.path:
        sys.path.insert(0, '/opt/trn_rl_repo')
    import concourse.bacc as bacc
    import concourse.tile as tile
    from concourse import mybir

    f32 = mybir.dt.float32
    bf16 = mybir.dt.float16
    ALU = mybir.AluOpType
    ACTF = mybir.ActivationFunctionType
    Q = TPC * CH
    EPC = TPC * T

    nc = bacc.Bacc("TRN2", target_bir_lowering=False, debug=False,
                   num_devices=NCORES)

    def dI(name, shape, dt=f32):
        return nc.dram_tensor(name, list(shape), dt, kind="ExternalInput")

    vcols_d = dI("vcols", (128, 3 * Q))
    rlo_d = dI("rlocols", (128, Q))
    rhi_d = dI("rhicols", (128, Q))
    slot_d = dI("slotrow", (1, EPC), bf16)
    zszr_d = dI("zszr", (64, EPC), bf16)
    wnames = [("we0", (72, 64)), ("we1", (64, 128)), ("we2", (128, 256)),
              ("we3a", (128, 64)), ("we3b", (128, 64)), ("wv96", (64, 96)),
              ("ww0", (64, 32)), ("ww1", (64, 32)),
              ("wm0x0", (64, 64)), ("wm0x1", (64, 64)),
              ("wm0s0", (96, 64)), ("wm0s1", (96, 64)),
              ("wm10", (64, 64)), ("wm11", (64, 64)),
              ("wm2", (64, 64)), ("wm2ro", (64, 1)), ("wv3", (96, 96)),
              ("y1sel", (3, 96)), ("ones64", (1, 64)),
              ("ident", (128, 128))]
    wd = {nm: dI(nm, sh, bf16) for nm, sh in wnames}
    cnames = [("iota64", (64, 1)), ("c128", (128, 128)), ("c79", (128, NHI)),
              ("id1f", (1, 1))]
    cd = {nm: dI(nm, sh, f32) for nm, sh in cnames}

    hist_d = nc.dram_tensor("hist", [NHI, 128], f32, kind="ExternalOutput")
    dbg_d = {}
    if debug:
        for nm, sh, dt in [("d_x0", (72, T), bf16), ("d_xe", (64, T), bf16),
                           ("d_indne", (NN, T), bf16), ("d_yrow", (4, T), bf16),
                           ("d_v", (96, T), bf16), ("d_sv", (96, T), bf16),
                           ("d_x1", (64, T), bf16), ("d_ee", (1, T), f32),
                           ("d_wys", (96, T), f32), ("d_eesc", (128, CH), f32)]:
            dbg_d[nm] = nc.dram_tensor(nm, list(sh), dt, kind="ExternalOutput")

    with tile.TileContext(nc) as tc:
        with tc.tile_pool(name="const", bufs=1) as cp, \
             tc.tile_pool(name="core", bufs=1) as cc, \
             tc.tile_pool(name="sbuf", bufs=3) as sb, \
             tc.tile_pool(name="ps_mm", bufs=2, space="PSUM") as ps_mm, \
             tc.tile_pool(name="ps_F", bufs=1, space="PSUM") as ps_F, \
             tc.tile_pool(name="ps_aux", bufs=2, space="PSUM") as ps_aux, \
             tc.tile_pool(name="ps_sm", bufs=2, space="PSUM") as ps_sm, \
             tc.tile_pool(name="ps_h", bufs=1, space="PSUM") as ps_h:

            W = {}
            for nm, sh in wnames:
                W[nm] = cp.tile(list(sh), bf16, name=nm, tag=nm)
                nc.sync.dma_start(out=W[nm][:], in_=wd[nm][:])
            C = {}
            for nm, sh in cnames:
                C[nm] = cp.tile(list(sh), f32, name=nm, tag=nm)
                nc.sync.dma_start(out=C[nm][:], in_=cd[nm][:])
            ident = W['ident']

            # ---------- per-core column-packed scalar phase ----------
            vcols = cc.tile([128, 3 * Q], f32)
            nc.sync.dma_start(out=vcols[:], in_=vcols_d[:])
            rloc = cc.tile([128, Q], f32)
            nc.sync.dma_start(out=rloc[:], in_=rlo_d[:])
            rhic = cc.tile([128, Q], f32)
            nc.sync.dma_start(out=rhic[:], in_=rhi_d[:])

            d2 = cc.tile([128, Q], f32)
            s0 = cc.tile([128, Q], f32)
            nc.vector.tensor_tensor(s0[:], vcols[:, 0:Q], vcols[:, 0:Q],
                                    op=ALU.mult)
            nc.vector.scalar_tensor_tensor(
                out=d2[:], in0=vcols[:, Q:2 * Q], scalar=1.0,
                in1=vcols[:, Q:2 * Q], op0=ALU.mult, op1=ALU.mult)
            nc.vector.tensor_tensor(d2[:], d2[:], s0[:], op=ALU.add)
            nc.vector.scalar_tensor_tensor(
                out=s0[:], in0=vcols[:, 2 * Q:3 * Q], scalar=1.0,
                in1=vcols[:, 2 * Q:3 * Q], op0=ALU.mult, op1=ALU.mult)
            nc.vector.tensor_tensor(d2[:], d2[:], s0[:], op=ALU.add)
            dd = cc.tile([128, Q], f32)
            nc.scalar.activation(dd[:], d2[:], ACTF.Sqrt)
            rd = cc.tile([128, Q], f32)
            nc.vector.reciprocal(rd[:], dd[:])
            d4 = cc.tile([128, Q], f32)
            nc.vector.tensor_tensor(d4[:], d2[:], d2[:], op=ALU.mult)
            d6 = cc.tile([128, Q], f32)
            nc.vector.tensor_tensor(d6[:], d4[:], d2[:], op=ALU.mult)
            p = cc.tile([128, Q], f32)
            nc.vector.tensor_scalar(p[:], dd[:], 48.0, -28.0,
                                    ALU.mult, ALU.add)
            nc.vector.scalar_tensor_tensor(out=p[:], in0=d2[:], scalar=-21.0,
                                           in1=p[:], op0=ALU.mult,
                                           op1=ALU.add)
            envc = cc.tile([128, Q], f32)
            nc.vector.tensor_tensor(envc[:], d6[:], p[:], op=ALU.mult)
            nc.vector.tensor_scalar(envc[:], envc[:], 1.0, None, ALU.add)
            envcb = cc.tile([128, Q], bf16)
            nc.scalar.copy(envcb[:], envc[:])
            env2c = cc.tile([128, Q], f32)
            nc.vector.tensor_tensor(env2c[:], envc[:], envc[:], op=ALU.mult)
            bsc = cc.tile([128, Q], f32)
            nc.vector.scalar_tensor_tensor(out=bsc[:], in0=envc[:],
                                           scalar=SQ2, in1=rd[:],
                                           op0=ALU.mult, op1=ALU.mult)
            sinc = cc.tile([128, N_RBF * Q], f32)
            # sin(n pi d) via Chebyshev recurrence (Sin LUT only valid ~[-pi,pi])
            hpi = cc.tile([128, 1], f32)
            nc.gpsimd.memset(hpi[:], float(np.pi / 2))
            c2 = cc.tile([128, Q], f32)
            nc.scalar.activation(c2[:], dd[:], ACTF.Sin, bias=hpi[:, 0:1],
                                 scale=float(-np.pi))
            nc.vector.tensor_scalar(c2[:], c2[:], 2.0, None, ALU.mult)
            nc.scalar.activation(sinc[:, 0:Q], dd[:], ACTF.Sin,
                                 scale=float(np.pi))
            nc.vector.tensor_tensor(sinc[:, Q:2 * Q], c2[:], sinc[:, 0:Q],
                                    op=ALU.mult)
            ctmp = cc.tile([128, Q], f32)
            for n in range(2, N_RBF):
                nc.vector.tensor_tensor(ctmp[:], c2[:],
                                        sinc[:, (n - 1) * Q:n * Q],
                                        op=ALU.mult)
                nc.vector.tensor_tensor(sinc[:, n * Q:(n + 1) * Q], ctmp[:],
                                        sinc[:, (n - 2) * Q:(n - 1) * Q],
                                        op=ALU.subtract)
            besc = cc.tile([128, N_RBF * Q], bf16)
            nc.vector.tensor_tensor(
                out=besc[:].rearrange("p (q n) -> p q n", n=N_RBF),
                in0=sinc[:].rearrange("p (n q) -> p q n", n=N_RBF),
                in1=bsc[:].rearrange("p (q o) -> p q o", o=1)
                    .to_broadcast([128, Q, N_RBF]),
                op=ALU.mult)
            yc = cc.tile([128, 4 * Q], bf16)
            nc.vector.memset(yc[:], 1.0)
            nc.vector.scalar_tensor_tensor(
                out=yc[:].rearrange("p (q c) -> p q c", c=4)[:, :, 0:3],
                in0=vcols[:].rearrange("p (j q) -> p q j", j=3),
                scalar=SQ3,
                in1=rd[:].rearrange("p (q o) -> p q o", o=1)
                    .to_broadcast([128, Q, 3]),
                op0=ALU.mult, op1=ALU.mult)

            hist_ps = ps_h.tile([NHI, 128], f32, space="PSUM", tag="hist")

            # ---------- per-tile loop ----------
            for t in range(TPC):
                esl = slice(T * t, T * (t + 1))

                x0 = sb.tile([72, T], bf16, tag="x0")
                x0r_ps = ps_sm.tile([8, T], bf16, space="PSUM", tag="sm")
                for k in range(CH):
                    q = CH * t + k
                    nc.tensor.transpose(
                        x0r_ps[:, 128 * k:128 * (k + 1)],
                        besc[:, N_RBF * q:N_RBF * (q + 1)], ident[:])
                nc.scalar.copy(x0[0:8, :], x0r_ps[:])
                nc.sync.dma_start(out=x0[8:72, :], in_=zszr_d[:, esl])

                yrow_ps = ps_sm.tile([4, T], bf16, space="PSUM", tag="sm")
                for k in range(CH):
                    q = CH * t + k
                    nc.tensor.transpose(
                        yrow_ps[:, 128 * k:128 * (k + 1)],
                        yc[:, 4 * q:4 * (q + 1)], ident[:])
                yrow = sb.tile([4, T], bf16, tag="yrow")
                nc.scalar.copy(yrow[:], yrow_ps[:])

                envr_ps = ps_sm.tile([1, T], bf16, space="PSUM", tag="sm")
                for k in range(CH):
                    q = CH * t + k
                    nc.tensor.transpose(
                        envr_ps[:, 128 * k:128 * (k + 1)],
                        envcb[:, q:q + 1], ident[:])
                envr = sb.tile([1, T], bf16, tag="envr")
                nc.scalar.copy(envr[:], envr_ps[:])
                envb_ps = ps_sm.tile([64, T], f32, space="PSUM", tag="sm")
                nc.tensor.matmul(envb_ps[:], W['ones64'][:], envr[:],
                                 start=True, stop=True)
                envb = sb.tile([64, T], f32, tag="envb")
                nc.scalar.copy(envb[:], envb_ps[:])

                slr = sb.tile([1, T], bf16, tag="slr")
                nc.sync.dma_start(out=slr[:], in_=slot_d[:, esl])
                slotb_ps = ps_sm.tile([NN, T], f32, space="PSUM", tag="sm")
                nc.tensor.matmul(slotb_ps[:], W['ones64'][:], slr[:],
                                 start=True, stop=True)
                ind_ne = sb.tile([NN, T], bf16, tag="ind_ne")
                nc.vector.tensor_scalar(ind_ne[:], slotb_ps[:],
                                        C['iota64'][:, 0:1], None,
                                        ALU.is_equal)
                indT_ps = ps_sm.tile([128, CH * NN], bf16, space="PSUM",
                                     tag="sm")
                for k in range(CH):
                    nc.tensor.transpose(
                        indT_ps[:, NN * k:NN * (k + 1)],
                        ind_ne[:, 128 * k:128 * (k + 1)], ident[0:NN, 0:NN])
                indT = sb.tile([128, CH * NN], bf16, tag="indT")
                nc.scalar.copy(indT[:], indT_ps[:])

                # ---- embedding MLP ----
                p1 = ps_mm.tile([64, T], f32, space="PSUM", tag="mm")
                nc.tensor.matmul(p1[:], W['we0'][:], x0[:],
                                 start=True, stop=True)
                h1 = sb.tile([64, T], bf16, tag="h1")
                nc.scalar.activation(h1[:], p1[:], ACTF.Silu)
                p2 = ps_mm.tile([128, T], f32, space="PSUM", tag="mm")
                nc.tensor.matmul(p2[:], W['we1'][:], h1[:],
                                 start=True, stop=True)
                h2 = sb.tile([128, T], bf16, tag="h2")
                nc.scalar.activation(h2[:], p2[:], ACTF.Silu)
                p3a = ps_mm.tile([128, T], f32, space="PSUM", tag="mm")
                nc.tensor.matmul(p3a[:], W['we2'][:, 0:128], h2[:],
                                 start=True, stop=True)
                h3a = sb.tile([128, T], bf16, tag="h3a")
                nc.scalar.activation(h3a[:], p3a[:], ACTF.Silu)
                p3b = ps_mm.tile([128, T], f32, space="PSUM", tag="mm")
                nc.tensor.matmul(p3b[:], W['we2'][:, 128:256], h2[:],
                                 start=True, stop=True)
                h3b = sb.tile([128, T], bf16, tag="h3b")
                nc.scalar.activation(h3b[:], p3b[:], ACTF.Silu)
                p4 = ps_mm.tile([64, T], f32, space="PSUM", tag="mm")
                nc.tensor.matmul(p4[:], W['we3a'][:], h3a[:],
                                 start=True, stop=False)
                nc.tensor.matmul(p4[:], W['we3b'][:], h3b[:],
                                 start=False, stop=True)
                x_sb = sb.tile([64, T], bf16, tag="x_sb")
                nc.vector.tensor_tensor(x_sb[:], p4[:], envb[:], op=ALU.mult)

                # ---- V init ----
                F0 = ps_F.tile([96, T], f32, space="PSUM", tag="F")
                nc.tensor.matmul(F0[:], W['wv96'][:], x_sb[:],
                                 start=True, stop=True)
                y1rep_ps = ps_aux.tile([96, T], f32, space="PSUM", tag="aux")
                nc.tensor.matmul(y1rep_ps[:], W['y1sel'][:], yrow[0:3, :],
                                 start=True, stop=True)
                y1rep = sb.tile([96, T], bf16, tag="y1rep")
                nc.scalar.copy(y1rep[:], y1rep_ps[:])
                V = sb.tile([96, T], bf16, tag="V")
                nc.vector.tensor_tensor(V[:], F0[:], y1rep[:], op=ALU.mult)

                x_cur = x_sb
                ee_sb = None
                for l in range(2):
                    w_ps = ps_mm.tile([32, T], f32, space="PSUM", tag="mm")
                    nc.tensor.matmul(w_ps[:], W[f'ww{l}'][:], x_cur[:],
                                     start=True, stop=True)
                    w_sb = sb.tile([32, T], bf16, tag="w_sb")
                    nc.scalar.copy(w_sb[:], w_ps[:])
                    wT_ps = ps_sm.tile([128, 128], bf16, space="PSUM",
                                       tag="sm")
                    for k in range(CH):
                        nc.tensor.transpose(
                            wT_ps[:, 32 * k:32 * (k + 1)],
                            w_sb[:, 128 * k:128 * (k + 1)],
                            ident[0:32, 0:32])
                    wyt = sb.tile([128, T], bf16, tag="wyt")
                    for k in range(CH):
                        q = CH * t + k
                        nc.vector.tensor_tensor(
                            out=wyt[:, 128 * k:128 * (k + 1)]
                                .rearrange("p (c m) -> p c m", c=4),
                            in0=wT_ps[:, 32 * k:32 * (k + 1)]
                                .rearrange("p (o m) -> p o m", o=1)
                                .to_broadcast([128, 4, 32]),
                            in1=yc[:, 4 * q:4 * (q + 1)]
                                .rearrange("p (c o) -> p c o", o=1)
                                .to_broadcast([128, 4, 32]),
                            op=ALU.mult)
                    S_ps = ps_sm.tile([NN, 128], f32, space="PSUM", tag="sm")
                    for k in range(CH):
                        nc.tensor.matmul(S_ps[:],
                                         indT[:, NN * k:NN * (k + 1)],
                                         wyt[:, 128 * k:128 * (k + 1)],
                                         start=(k == 0), stop=(k == CH - 1))
                    S_sb = sb.tile([NN, 96], bf16, tag="S_sb")
                    nc.scalar.mul(S_sb[:], S_ps[:, 0:96], float(eps))
                    wys_ps = ps_aux.tile([96, T], f32, space="PSUM",
                                         tag="aux")
                    nc.tensor.matmul(wys_ps[:], S_sb[:], ind_ne[:],
                                     start=True, stop=True)
                    sv = sb.tile([96, T], bf16, tag="sv")
                    nc.vector.tensor_tensor(sv[:], wys_ps[:], V[:],
                                            op=ALU.mult)
                    m0 = ps_mm.tile([64, T], f32, space="PSUM", tag="mm")
                    nc.tensor.matmul(m0[:], W[f'wm0x{l}'][:], x_cur[:],
                                     start=True, stop=False,
                                     skip_group_check=True)
                    nc.tensor.matmul(m0[:], W[f'wm0s{l}'][:], sv[:],
                                     start=False, stop=True,
                                     skip_group_check=True)
                    mh0 = sb.tile([64, T], bf16, tag="mh0")
                    nc.scalar.activation(mh0[:], m0[:], ACTF.Silu)
                    m1 = ps_mm.tile([64, T], f32, space="PSUM", tag="mm")
                    nc.tensor.matmul(m1[:], W[f'wm1{l}'][:], mh0[:],
                                     start=True, stop=True)
                    mh1 = sb.tile([64, T], bf16, tag="mh1")
                    nc.scalar.activation(mh1[:], m1[:], ACTF.Silu)
                    if l == 0:
                        S_a3 = sb.tile([NN, 96], bf16, tag="S_a3")
                        nc.scalar.mul(
                            S_a3[:].rearrange("p (o m) -> p o m", o=3),
                            S_ps[:, 96:128].rearrange("p (o m) -> p o m", o=1)
                                .to_broadcast([NN, 3, 32]),
                            float(eps))
                        arep_ps = ps_aux.tile([96, T], f32, space="PSUM",
                                              tag="aux")
                        nc.tensor.matmul(arep_ps[:], S_a3[:], ind_ne[:],
                                         start=True, stop=True)
                        vout = sb.tile([96, T], bf16, tag="vout")
                        nc.vector.tensor_tensor(vout[:], arep_ps[:], V[:],
                                                op=ALU.mult)
                        v1_ps = ps_aux.tile([96, T], f32, space="PSUM",
                                            tag="aux")
                        nc.tensor.matmul(v1_ps[:], W['wv3'][:], vout[:],
                                         start=True, stop=True)
                        V = sb.tile([96, T], bf16, tag="V1")
                        nc.scalar.copy(V[:], v1_ps[:])
                        m2 = ps_mm.tile([64, T], f32, space="PSUM", tag="mm")
                        nc.tensor.matmul(m2[:], W['wm2'][:], mh1[:],
                                         start=True, stop=True)
                        x1_sb = sb.tile([64, T], bf16, tag="x1_sb")
                        nc.vector.tensor_tensor(x1_sb[:], m2[:], envb[:],
                                                op=ALU.mult)
                        if debug and t == 0:
                            wys_dbg = sb.tile([96, T], f32, tag="wys_dbg")
                            nc.scalar.copy(wys_dbg[:], wys_ps[:])
                            nc.sync.dma_start(out=dbg_d['d_wys'][:],
                                              in_=wys_dbg[:])
                            nc.sync.dma_start(out=dbg_d['d_sv'][:], in_=sv[:])
                        x_cur = x1_sb
                    else:
                        ee_ps = ps_mm.tile([1, T], f32, space="PSUM",
                                           tag="mm")
                        nc.tensor.matmul(ee_ps[:], W['wm2ro'][:], mh1[:],
                                         start=True, stop=True)
                        ee_sb = sb.tile([1, T], f32, tag="ee_sb")
                        nc.scalar.copy(ee_sb[:], ee_ps[:])

                eec_ps = ps_sm.tile([128, CH], f32, space="PSUM", tag="sm")
                for k in range(CH):
                    nc.tensor.transpose(eec_ps[:, k:k + 1],
                                        ee_sb[:, 128 * k:128 * (k + 1)],
                                        C['id1f'][:])
                eesc = sb.tile([128, CH], f32, tag="eesc")
                nc.vector.tensor_tensor(eesc[:], eec_ps[:],
                                        env2c[:, CH * t:CH * (t + 1)],
                                        op=ALU.mult)
                for k in range(CH):
                    q = CH * t + k
                    ohlo = sb.tile([128, 128], bf16, tag="ohlo")
                    nc.vector.scalar_tensor_tensor(
                        out=ohlo[:], in0=C['c128'][:],
                        scalar=rloc[:, q:q + 1],
                        in1=eesc[:, k:k + 1].to_broadcast([128, 128]),
                        op0=ALU.is_equal, op1=ALU.mult)
                    ohhi = sb.tile([128, NHI], bf16, tag="ohhi")
                    nc.vector.tensor_scalar(ohhi[:], C['c79'][:],
                                            rhic[:, q:q + 1], None,
                                            ALU.is_equal)
                    nc.tensor.matmul(hist_ps[:], ohhi[:], ohlo[:],
                                     start=(t == 0 and k == 0),
                                     stop=(t == TPC - 1 and k == CH - 1),
                                     skip_group_check=True)

                if debug and t == 0:
                    nc.sync.dma_start(out=dbg_d['d_x0'][:], in_=x0[:])
                    nc.sync.dma_start(out=dbg_d['d_xe'][:], in_=x_sb[:])
                    nc.sync.dma_start(out=dbg_d['d_indne'][:], in_=ind_ne[:])
                    nc.sync.dma_start(out=dbg_d['d_yrow'][:], in_=yrow[:])
                    nc.sync.dma_start(out=dbg_d['d_v'][:], in_=V[:])
                    nc.sync.dma_start(out=dbg_d['d_x1'][:], in_=x1_sb[:])
                    nc.sync.dma_start(out=dbg_d['d_ee'][:], in_=ee_sb[:])
                    nc.sync.dma_start(out=dbg_d['d_eesc'][:], in_=eesc[:])

            hist_sb = sb.tile([NHI, 128], f32, tag="hist_sb")
            nc.scalar.copy(hist_sb[:], hist_ps[:])
            nc.sync.dma_start(out=hist_d[:], in_=hist_sb[:])

    nc.compile()
    return nc


# ---------------------------------------------------------------------------
# Entry point
# ---------------------------------------------------------------------------
LAST_EXEC_NS = None


def kernel(vectors, senders, receivers, species, emb_species,
           W_e0, W_e1, W_e2, W_e3, W_wvec, W_vinit,
           W_w, W_m0, W_m1, W_m2, W_V, W_r0, W_rout,
           particle_energy, varepsilon, _trace=False):
    global LAST_EXEC_NS
    import sys
    if '/opt/trn_rl_repo' not in sys.path:
        sys.path.insert(0, '/opt/trn_rl_repo')
    from concourse.bass_utils import run_bass_kernel_spmd

    vectors = np.asarray(vectors, np.float32)
    senders = np.asarray(senders)
    receivers = np.asarray(receivers)
    species = np.asarray(species)
    emb_species = np.asarray(emb_species, np.float32)
    inp = dict(W_e0=W_e0, W_e1=W_e1, W_e2=W_e2, W_e3=W_e3, W_wvec=W_wvec,
               W_vinit=W_vinit, W_w=W_w, W_m0=W_m0, W_m1=W_m1, W_m2=W_m2,
               W_V=W_V, W_r0=W_r0, W_rout=W_rout)

    sp = np.log1p(np.exp(np.float32(varepsilon)))
    eps = float(np.float32(1.0) / np.sqrt(np.float32(1.0) + sp))

    node_emb = emb_species[species]                 # [N, 32] table gather
    cores, TPC = _pack(vectors, senders, receivers, node_emb)
    wts, consts = _prep_weights(inp)

    nc = _build(TPC, eps, debug=False)

    in_maps = []
    for c in range(NCORES):
        m = {}
        m.update({k: v for k, v in cores[c].items() if not k.startswith('_')})
        m.update(wts)
        m.update(consts)
        in_maps.append(m)

    res = run_bass_kernel_spmd(nc, in_maps, list(range(NCORES)),
                               trace=_trace)
    LAST_EXEC_NS = res.exec_time_ns

    # unshard: sum partial receiver histograms, add per-species energy
    total = np.zeros((NHI * 128,), np.float32)
    for c in range(NCORES):
        total += res.results[c]['hist'].reshape(-1).astype(np.float32)
    node_e = total[:N_NODES, None] + np.asarray(
        particle_energy, np.float32)[species]
    return node_e.astype(np.float32)


if __name__ == "__main__":
    pass
